# revision 18
# baseline (speedup 1.0000x reference)
"""Bayesian RNN (BRNN) Trainium2 kernel.

Data-parallel over batch: each of 8 NeuronCores handles 16 of the 128
batch samples.  The per-sample sampled weight W[b] = W_mu + sigma*eps[b]
(1024x1025 each) is built on-device, transposed, quantized to fp8-e4m3
and kept fully resident in SBUF (16.8 MB/core), so the 255-step
recurrence runs with zero HBM traffic for the weights.  The recurrent
loop computes the GRU cell + per-sample matvec on the tensor engine
(fp8 stationaries, fp32 PSUM accumulation); the decoder / emission /
log-likelihood stage is deferred and done as dense matmuls after the
scan.  KL is computed in fp32 on every core (identical values); the
host sums the 8 per-core LL partials.

Numerics validated against the fp32 reference scheme: fp8 weights with
fp16 master activations give ~6e-4 relative error on the final scalars
(tolerance 2e-2).
"""

import numpy as np
import ml_dtypes
import orjson

import concourse.bass as bass
import concourse.mybir as mybir
import concourse.tile as tile
import concourse.bass2jax as bass2jax
from concourse.bass_utils import (
    run_bass_kernel_spmd,
    compile_bir_kernel as _orig_compile_bir_kernel,
)

H = 1024
OUT = 32
ACTD = 8
B = 128
SEQ = 256
SM1 = SEQ - 1          # 255 recurrent steps
NC = 8                 # cores
BC = B // NC           # 16 batch samples per core
KT = H // 128          # 8 h-tiles
MT3 = 3 * H // 128     # 24 gru output tiles
LOG2PI = float(np.log(2.0 * np.pi))

F32 = mybir.dt.float32
F16 = mybir.dt.float16
F8 = mybir.dt.float8e4
AF = mybir.ActivationFunctionType
ALU = mybir.AluOpType
PSUM = bass.MemorySpace.PSUM

NP_F8 = ml_dtypes.float8_e4m3


# ---------------------------------------------------------------------------
# walrus on this container caps sync-waits at 1 per instruction; tile's
# kernel-tail drain (and some barriers) carry several.  Split extra waits
# onto same-engine NoOps ahead of the instruction (order-preserving, so
# semantically identical).
def _split_waits(bir):
    n = 0
    for f in bir["functions"]:
        for blk in f["blocks"]:
            out_insts = []
            for ins in blk["instructions"]:
                si = ins.get("sync_info")
                if si and len(si.get("on_wait", [])) > 1:
                    waits = si["on_wait"]
                    for w in waits[:-1]:
                        n += 1
                        out_insts.append({
                            "engine": ins["engine"],
                            "ins": [], "outs": [],
                            "name": f"I-waitsplit-{n}",
                            "opcode": "NoOp",
                            "sync_info": {"on_update": [], "on_wait": [w]},
                        })
                    si["on_wait"] = waits[-1:]
                out_insts.append(ins)
            blk["instructions"] = out_insts
    return bir


def _patched_compile(bir_json, tmpdir, neff_name="file.neff"):
    bir = _split_waits(orjson.loads(bir_json))
    return _orig_compile_bir_kernel(orjson.dumps(bir), tmpdir, neff_name)


def _enable_ldw_opt():
    import concourse.bass_utils as bu
    orig = bu.bir_verify_and_optimise

    def patched(tmpdir, inp="bir.json", outp="file.neff", arch=None, *,
                dve_root=None):
        import subprocess
        real_run = bu.run_command

        def run_hook(argv, **kw):
            argv = [a.replace("--enable-ldw-opt=false",
                              "--enable-ldw-opt=true") for a in argv]
            return real_run(argv, **kw)
        bu.run_command = run_hook
        try:
            return orig(tmpdir, inp, outp, arch, dve_root=dve_root)
        finally:
            bu.run_command = real_run
    bu.bir_verify_and_optimise = patched


bass2jax.compile_bir_kernel = _patched_compile


# ---------------------------------------------------------------------------
def build_nc(T=SM1, no_bmm=False, no_gru=False, reps=1):
    nc = bass.Bass("TRN2", target_bir_lowering=False, debug=False,
                   num_devices=NC)

    def inp(name, shape, dt):
        return nc.declare_dram_parameter(name, list(shape), dt, isOutput=False)

    eps_in = inp("eps", [BC, H, H + 1], F32)
    at_in = inp("a_t", [ACTD + 1, SM1, BC], F8)
    x0_in = inp("x0", [OUT, BC], F16)
    xt_in = inp("xt", [OUT, SM1 * BC], F32)
    wmu_in = inp("wmu32", [H, H + 1], F32)
    sig_in = inp("sig32", [H, H + 1], F32)
    whh_in = inp("whh8", [128, KT * MT3 * 128], F8)
    wih_in = inp("wih8", [ACTD + 1, 3 * H], F8)
    bhhn_in = inp("bhhn8", [1, H], F8)
    ones_in = inp("ones8", [1, BC], F8)
    ew1_in = inp("ew1", [OUT, H], F16)
    eb1_in = inp("eb1", [128, KT], F32)
    ew2_in = inp("ew2l", [128, KT * H], F16)
    eb2_in = inp("eb2", [128, KT], F32)
    dw_in = inp("dwl", [128, KT * H], F16)
    db_in = inp("db", [128, KT], F32)
    emw_in = inp("emwl", [128, KT * OUT], F16)
    evw_in = inp("evwl", [128, KT * OUT], F16)
    evb_in = inp("evb", [OUT, 1], F32)
    id_in = inp("ident", [128, 128], F32)
    kllv_in = inp("kl_lv", [H, H + 1], F32)
    klpm_in = inp("kl_pm", [H, H + 1], F32)
    klpl_in = inp("kl_pl", [H, H + 1], F32)
    out_ext = nc.declare_dram_parameter("out", [1, 2], F32, isOutput=True)

    nh_dram = nc.dram_tensor("nh_all", [SM1, 128 * 128], F16)

    with tile.TileContext(nc) as tc:
        _rep = tc.For_i(0, reps, 1) if reps > 1 else None
        if _rep is not None:
            _rep.__enter__()
        with tc.tile_pool(name="main", bufs=1) as mp:
            # ---- persistent (through the recurrent loop) tiles ----
            wt_sb = mp.tile([128, BC * KT * H], F8)      # W_T resident
            whh_sb = mp.tile([128, KT * MT3 * 128], F8)
            wih_sb = mp.tile([ACTD + 1, 3 * H], F8)
            a_sb = mp.tile([ACTD + 1, SM1, BC], F8)
            bhhn_sb = mp.tile([1, H], F8)
            ones_sb = mp.tile([1, BC], F8)
            bias_sb = mp.tile([128, 128], F32)           # bmm bias, (it,b)
            h_sb = mp.tile([128, 128], F16)              # carry, (kt,b)
            ident_sb = mp.tile([128, 128], F32)

            nc.sync.dma_start(out=whh_sb[:, :], in_=whh_in.ap()[:, :])
            nc.sync.dma_start(out=wih_sb[:, :], in_=wih_in.ap()[:, :])
            nc.sync.dma_start(out=a_sb[:, :, :], in_=at_in.ap()[:, :, :])
            nc.sync.dma_start(out=bhhn_sb[:, :], in_=bhhn_in.ap()[:, :])
            nc.sync.dma_start(out=ones_sb[:, :], in_=ones_in.ap()[:, :])
            nc.sync.dma_start(out=ident_sb[:, :], in_=id_in.ap()[:, :])

            # ---- initial encoder: h0 = tanh(relu(x0@w1.T+b1)@w2.T+b2) ----
            with tc.tile_pool(name="enc", bufs=1) as ep, \
                 tc.tile_pool(name="encps", bufs=2, space=PSUM) as eps_ps:
                ew1_sb = ep.tile([OUT, H], F16)
                eb1_sb = ep.tile([128, KT], F32)
                ew2_sb = ep.tile([128, KT * H], F16)
                eb2_sb = ep.tile([128, KT], F32)
                x0_sb = ep.tile([OUT, BC], F16)
                e1_sb = ep.tile([128, KT * BC], F16)
                nc.sync.dma_start(out=ew1_sb[:, :], in_=ew1_in.ap()[:, :])
                nc.sync.dma_start(out=eb1_sb[:, :], in_=eb1_in.ap()[:, :])
                nc.sync.dma_start(out=ew2_sb[:, :], in_=ew2_in.ap()[:, :])
                nc.sync.dma_start(out=eb2_sb[:, :], in_=eb2_in.ap()[:, :])
                nc.sync.dma_start(out=x0_sb[:, :], in_=x0_in.ap()[:, :])
                for mt in range(KT):
                    ps = eps_ps.tile([128, BC], F32, tag="encp")
                    nc.tensor.matmul(ps[:, :],
                                     ew1_sb[:, mt * 128:(mt + 1) * 128],
                                     x0_sb[:, :], start=True, stop=True)
                    nc.scalar.activation(e1_sb[:, mt * BC:(mt + 1) * BC],
                                         ps[:, :], AF.Relu,
                                         bias=eb1_sb[:, mt:mt + 1])
                for mt in range(KT):
                    ps = eps_ps.tile([128, BC], F32, tag="encp")
                    for kt in range(KT):
                        nc.tensor.matmul(
                            ps[:, :],
                            ew2_sb[:, kt * H + mt * 128: kt * H + (mt + 1) * 128],
                            e1_sb[:, kt * BC:(kt + 1) * BC],
                            start=(kt == 0), stop=(kt == KT - 1))
                    nc.scalar.activation(h_sb[:, mt * BC:(mt + 1) * BC],
                                         ps[:, :], AF.Tanh,
                                         bias=eb2_sb[:, mt:mt + 1])

            # ---- W transform: W_T[b] = (W_mu + sigma*eps[b]).T -> fp8 ----
            with tc.tile_pool(name="xf", bufs=1) as xp, \
                 tc.tile_pool(name="xfps", bufs=4, space=PSUM) as xps:
                for it in range(KT):
                    mu_t = xp.tile([128, H + 1], F32, tag="mu")
                    sg_t = xp.tile([128, H + 1], F32, tag="sg")
                    nc.sync.dma_start(
                        out=mu_t[:, :],
                        in_=wmu_in.ap()[it * 128:(it + 1) * 128, :])
                    nc.sync.dma_start(
                        out=sg_t[:, :],
                        in_=sig_in.ap()[it * 128:(it + 1) * 128, :])
                    for b in range(BC):
                        et = xp.tile([128, H + 1], F32, tag="eps")
                        nc.sync.dma_start(
                            out=et[:, :],
                            in_=eps_in.ap()[b, it * 128:(it + 1) * 128, :])
                        w32 = xp.tile([128, H], F32, tag="w32")
                        nc.vector.tensor_tensor(w32[:, :], et[:, :H],
                                                sg_t[:, :H], op=ALU.mult)
                        nc.vector.tensor_tensor(w32[:, :], w32[:, :],
                                                mu_t[:, :H], op=ALU.add)
                        nc.vector.scalar_tensor_tensor(
                            bias_sb[:, it * BC + b: it * BC + b + 1],
                            et[:, H:H + 1], sg_t[:, H:H + 1],
                            mu_t[:, H:H + 1], op0=ALU.mult, op1=ALU.add)
                        for jt in range(KT):
                            tp = xps.tile([128, 128], F32, tag="tp")
                            nc.tensor.transpose(
                                tp[:, :], w32[:, jt * 128:(jt + 1) * 128],
                                ident_sb[:, :])
                            col = (b * KT + jt) * H + it * 128
                            nc.scalar.copy(wt_sb[:, col:col + 128], tp[:, :])

            # ---- recurrent loop (3-step unrolled) ----
            UNR = 3 if T % 3 == 0 else 1
            a_re = a_sb[:, :, :].rearrange("k (g u) b -> k g u b", u=UNR)
            nh_re_st = nh_dram.ap().rearrange("(g u) f -> g u f", u=UNR)
            with tc.tile_pool(name="lp", bufs=2) as lp, \
                 tc.tile_pool(name="lps", bufs=2, space=PSUM) as lps:
                with tc.For_i(0, T // UNR, 1,
                              hint_engines=(mybir.EngineType.PE,)) as i:
                  a_u = lp.tile([ACTD + 1, UNR, BC], F8, tag="a_u")
                  nc.vector.tensor_copy(a_u[:, :, :],
                                        a_re[:, bass.ds(i, 1), :, :])
                  for uu in range(UNR):
                    ghrz = lps.tile([128, 16 * BC], F32, tag="ghrz")
                    ghn = lps.tile([128, KT * BC], F32, tag="ghn")
                    gin = lps.tile([128, KT * BC], F32, tag="gin")
                    a_t = a_u[:, uu, :]
                    nkt = 1 if no_gru else KT
                    for mt in range(MT3):
                        dst = (ghrz[:, mt * BC:(mt + 1) * BC] if mt < 16
                               else ghn[:, (mt - 16) * BC:(mt - 15) * BC])
                        for kt in range(nkt):
                            nc.tensor.matmul(
                                dst,
                                whh_sb[:, (kt * MT3 + mt) * 128:
                                       (kt * MT3 + mt + 1) * 128],
                                h_sb[:, kt * BC:(kt + 1) * BC],
                                start=(kt == 0), stop=False)
                        if mt < 16:
                            # gi + (b_ih+b_hh) ride the K=9 augmented row
                            nc.tensor.matmul(
                                dst,
                                wih_sb[:, mt * 128:(mt + 1) * 128],
                                a_t, start=False, stop=True)
                        else:
                            # n-gate hh side: + b_hh_n via K=1 ones matmul
                            nc.tensor.matmul(
                                dst,
                                bhhn_sb[:, (mt - 16) * 128:(mt - 15) * 128],
                                ones_sb[:, :], start=False, stop=True)
                    for mt in range(KT):
                        # gi_n + b_ih_n (augmented row)
                        nc.tensor.matmul(
                            gin[:, mt * BC:(mt + 1) * BC],
                            wih_sb[:, (16 + mt) * 128:(17 + mt) * 128],
                            a_t, start=True, stop=True)

                    # sigmoid(x) = 0.5*tanh(x/2)+0.5 -- keeps ACT on the
                    # tanh table all loop long (no table reloads)
                    rz_sb = lp.tile([128, 256], F32, tag="rz")
                    nc.scalar.activation(rz_sb[:, :], ghrz[:, :], AF.Tanh,
                                         scale=0.5)
                    t1 = lp.tile([128, 128], F32, tag="t1")
                    nc.vector.scalar_tensor_tensor(
                        t1[:, :], rz_sb[:, 0:128], 1.0, ghn[:, :],
                        op0=ALU.add, op1=ALU.mult)
                    t2 = lp.tile([128, 128], F32, tag="t2")
                    nc.vector.scalar_tensor_tensor(
                        t2[:, :], t1[:, :], 0.5, gin[:, :],
                        op0=ALU.mult, op1=ALU.add)
                    n_sb = lp.tile([128, 128], F16, tag="n")
                    nc.scalar.activation(n_sb[:, :], t2[:, :], AF.Tanh)
                    d_sb = lp.tile([128, 128], F32, tag="d")
                    nc.vector.tensor_tensor(d_sb[:, :], h_sb[:, :],
                                            n_sb[:, :], op=ALU.subtract)
                    zd_sb = lp.tile([128, 128], F32, tag="zd")
                    nc.vector.scalar_tensor_tensor(
                        zd_sb[:, :], rz_sb[:, 128:256], 1.0, d_sb[:, :],
                        op0=ALU.add, op1=ALU.mult)
                    hc_sb = lp.tile([128, 128], F16, tag="hc")
                    nc.vector.scalar_tensor_tensor(
                        hc_sb[:, :], zd_sb[:, :], 0.5, n_sb[:, :],
                        op0=ALU.mult, op1=ALU.add)

                    g_ps = lps.tile([128, 128], F32, tag="g")
                    if not no_bmm:
                        for b in range(BC):
                            for it in range(KT):
                                col = it * BC + b
                                for jt in range(KT):
                                    w0 = (b * KT + jt) * H + it * 128
                                    nc.tensor.matmul(
                                        g_ps[:, col:col + 1],
                                        wt_sb[:, w0:w0 + 128],
                                        hc_sb[:, jt * BC + b:jt * BC + b + 1],
                                        start=(jt == 0), stop=(jt == KT - 1))
                    else:
                        nc.tensor.matmul(g_ps[:, :], wt_sb[:, 0:128],
                                         hc_sb[:, :], start=True, stop=True)
                    g2 = lp.tile([128, 128], F32, tag="g2")
                    nc.vector.tensor_tensor(g2[:, :], g_ps[:, :],
                                            bias_sb[:, :], op=ALU.add)
                    nc.scalar.activation(h_sb[:, :], g2[:, :], AF.Tanh)
                    nc.sync.dma_start(out=nh_re_st[bass.ds(i, 1), uu, :],
                                      in_=h_sb[:, :])

        # ---- post pass: decoder, emission, log-likelihood, KL ----
        ROWS = T * BC
        NCH = (ROWS + 509) // 510
        with tc.tile_pool(name="post", bufs=1) as pp, \
             tc.tile_pool(name="postps", bufs=2, space=PSUM) as pps:
            dw_sb = pp.tile([128, KT * H], F16)
            db_sb = pp.tile([128, KT], F32)
            emw_sb = pp.tile([128, KT * OUT], F16)
            evw_sb = pp.tile([128, KT * OUT], F16)
            evb_sb = pp.tile([OUT, 1], F32)
            xt_sb = pp.tile([OUT, SM1 * BC], F32)
            acc_q = pp.tile([OUT, NCH], F32)
            acc_lv = pp.tile([OUT, NCH], F32)
            out_sb = pp.tile([1, 2], F32)
            nc.sync.dma_start(out=dw_sb[:, :], in_=dw_in.ap()[:, :])
            nc.sync.dma_start(out=db_sb[:, :], in_=db_in.ap()[:, :])
            nc.sync.dma_start(out=emw_sb[:, :], in_=emw_in.ap()[:, :])
            nc.sync.dma_start(out=evw_sb[:, :], in_=evw_in.ap()[:, :])
            nc.sync.dma_start(out=evb_sb[:, :], in_=evb_in.ap()[:, :])
            nc.sync.dma_start(out=xt_sb[:, :T * BC], in_=xt_in.ap()[:, :T * BC])

            nh_t = []
            nh_re = nh_dram.ap().rearrange("t (p kt b) -> p kt t b",
                                           p=128, kt=KT, b=BC)
            for kt in range(KT):
                t_ = pp.tile([128, ROWS], F16, tag=f"nh{kt}")
                nc.sync.dma_start(out=t_[:, :],
                                  in_=nh_re[:, kt, :T, :])
                nh_t.append(t_)

            p2_ctx = tc.tile_pool(name="post2", bufs=2)
            p2 = p2_ctx.__enter__()
            for rc in range(NCH):
                r0 = rc * 510
                rn = min(510, ROWS - r0)
                dec_t = []
                for mt in range(KT):
                    ps = pps.tile([128, 510], F32, tag="decp")
                    for kt in range(KT):
                        nc.tensor.matmul(
                            ps[:, :rn],
                            dw_sb[:, kt * H + mt * 128: kt * H + (mt + 1) * 128],
                            nh_t[kt][:, r0:r0 + rn],
                            start=(kt == 0), stop=(kt == KT - 1))
                    dt_ = p2.tile([128, 510], F16, tag=f"dec{mt}")
                    nc.scalar.activation(dt_[:, :rn], ps[:, :rn], AF.Relu,
                                         bias=db_sb[:, mt:mt + 1])
                    dec_t.append(dt_)
                m_ps = pps.tile([OUT, 510], F32, tag="mp")
                v_ps = pps.tile([OUT, 510], F32, tag="vp")
                for kt in range(KT):
                    nc.tensor.matmul(m_ps[:, :rn],
                                     emw_sb[:, kt * OUT:(kt + 1) * OUT],
                                     dec_t[kt][:, :rn],
                                     start=(kt == 0), stop=(kt == KT - 1))
                for kt in range(KT):
                    nc.tensor.matmul(v_ps[:, :rn],
                                     evw_sb[:, kt * OUT:(kt + 1) * OUT],
                                     dec_t[kt][:, :rn],
                                     start=(kt == 0), stop=(kt == KT - 1))
                # e = exp(-(v + evb)) ; d = xt' - m ; q = d*d*e
                e_sb = p2.tile([OUT, 510], F32, tag="e")
                nc.scalar.activation(e_sb[:, :rn], v_ps[:, :rn], AF.Exp,
                                     bias=evb_sb[:, 0:1], scale=-1.0)
                d_sb2 = p2.tile([OUT, 510], F32, tag="dd")
                nc.vector.tensor_tensor(d_sb2[:, :rn], xt_sb[:, r0:r0 + rn],
                                        m_ps[:, :rn], op=ALU.subtract)
                d2_sb = p2.tile([OUT, 510], F32, tag="d2")
                nc.vector.tensor_tensor(d2_sb[:, :rn], d_sb2[:, :rn],
                                        d_sb2[:, :rn], op=ALU.mult)
                q_sb = p2.tile([OUT, 510], F32, tag="q")
                nc.vector.scalar_tensor_tensor(
                    q_sb[:, :rn], d2_sb[:, :rn], 1.0, e_sb[:, :rn],
                    op0=ALU.mult, op1=ALU.mult,
                    accum_out=acc_q[:, rc:rc + 1])
                nc.vector.tensor_reduce(acc_lv[:, rc:rc + 1], v_ps[:, :rn],
                                        axis=mybir.AxisListType.X, op=ALU.add)

            p2_ctx.__exit__(None, None, None)

            # KL on every core (identical) in fp32
            acc_kl = pp.tile([128, KT], F32)
            p2_ctx = tc.tile_pool(name="klp", bufs=1)
            p2 = p2_ctx.__enter__()
            for it in range(KT):
                lv_t = p2.tile([128, H + 1], F32, tag="klv")
                pm_t = p2.tile([128, H + 1], F32, tag="kpm")
                pl_t = p2.tile([128, H + 1], F32, tag="kpl")
                mu_t = p2.tile([128, H + 1], F32, tag="kmu")
                sl = slice(it * 128, (it + 1) * 128)
                nc.sync.dma_start(out=lv_t[:, :], in_=kllv_in.ap()[sl, :])
                nc.sync.dma_start(out=pm_t[:, :], in_=klpm_in.ap()[sl, :])
                nc.sync.dma_start(out=pl_t[:, :], in_=klpl_in.ap()[sl, :])
                nc.sync.dma_start(out=mu_t[:, :], in_=wmu_in.ap()[sl, :])
                eq = p2.tile([128, H + 1], F32, tag="keq")
                nc.scalar.activation(eq[:, :], lv_t[:, :], AF.Exp)
                ep = p2.tile([128, H + 1], F32, tag="kep")
                nc.scalar.activation(ep[:, :], pl_t[:, :], AF.Exp, scale=-1.0)
                dmu = p2.tile([128, H + 1], F32, tag="kdm")
                nc.vector.tensor_tensor(dmu[:, :], mu_t[:, :], pm_t[:, :],
                                        op=ALU.subtract)
                d2m = p2.tile([128, H + 1], F32, tag="kd2")
                nc.vector.tensor_tensor(d2m[:, :], dmu[:, :], dmu[:, :],
                                        op=ALU.mult)
                s_ = p2.tile([128, H + 1], F32, tag="ks")
                nc.vector.tensor_tensor(s_[:, :], eq[:, :], d2m[:, :],
                                        op=ALU.add)
                t1_ = p2.tile([128, H + 1], F32, tag="kt1")
                nc.vector.tensor_tensor(t1_[:, :], s_[:, :], ep[:, :],
                                        op=ALU.mult)
                v1_ = p2.tile([128, H + 1], F32, tag="kv1")
                nc.vector.tensor_tensor(v1_[:, :], pl_t[:, :], lv_t[:, :],
                                        op=ALU.subtract)
                w_ = p2.tile([128, H + 1], F32, tag="kw")
                nc.vector.tensor_tensor(w_[:, :], v1_[:, :], t1_[:, :],
                                        op=ALU.add)
                kls = p2.tile([128, H + 1], F32, tag="kls")
                nc.vector.scalar_tensor_tensor(
                    kls[:, :], w_[:, :], -1.0, w_[:, :],
                    op0=ALU.add, op1=ALU.bypass,
                    accum_out=acc_kl[:, it:it + 1])

            p2_ctx.__exit__(None, None, None)

            # reductions to scalars
            s1a = pp.tile([OUT, 1], F32)
            nc.vector.tensor_reduce(s1a[:, :], acc_q[:, :],
                                    axis=mybir.AxisListType.X, op=ALU.add)
            s2a = pp.tile([OUT, 1], F32)
            nc.vector.tensor_reduce(s2a[:, :], acc_lv[:, :],
                                    axis=mybir.AxisListType.X, op=ALU.add)
            ka = pp.tile([128, 1], F32)
            nc.vector.tensor_reduce(ka[:, :], acc_kl[:, :],
                                    axis=mybir.AxisListType.X, op=ALU.add)
            s1 = pp.tile([1, 1], F32)
            nc.gpsimd.tensor_reduce(s1[:, :], s1a[:, :],
                                    axis=mybir.AxisListType.C, op=ALU.add)
            s2 = pp.tile([1, 1], F32)
            nc.gpsimd.tensor_reduce(s2[:, :], s2a[:, :],
                                    axis=mybir.AxisListType.C, op=ALU.add)
            sb_ = pp.tile([1, 1], F32)
            nc.gpsimd.tensor_reduce(sb_[:, :], evb_sb[:, :],
                                    axis=mybir.AxisListType.C, op=ALU.add)
            k1 = pp.tile([1, 1], F32)
            nc.gpsimd.tensor_reduce(k1[:, :], ka[:, :],
                                    axis=mybir.AxisListType.C, op=ALU.add)
            # LL = -0.5*(S1 + S2 + ROWS*SB + LOG2PI*NE)
            ne = float(ROWS * OUT)
            u1 = pp.tile([1, 1], F32)
            nc.vector.tensor_tensor(u1[:, :], s1[:, :], s2[:, :], op=ALU.add)
            u2 = pp.tile([1, 1], F32)
            nc.vector.scalar_tensor_tensor(u2[:, :], sb_[:, :], float(ROWS),
                                           u1[:, :], op0=ALU.mult, op1=ALU.add)
            u3 = pp.tile([1, 1], F32)
            nc.vector.scalar_tensor_tensor(u3[:, :], u2[:, :],
                                           LOG2PI * ne, u2[:, :],
                                           op0=ALU.add, op1=ALU.bypass)
            nc.vector.tensor_scalar_mul(out_sb[:, 0:1], u3[:, :], -0.5)
            nc.vector.tensor_scalar_mul(out_sb[:, 1:2], k1[:, :], 0.5)
            nc.sync.dma_start(out=out_ext.ap()[:, :], in_=out_sb[:, :])
        if _rep is not None:
            _rep.__exit__(None, None, None)

    return nc


# ---------------------------------------------------------------------------
def prep_inputs(X, A, W_eps, W_mu, W_logvar, prior_W_mu, prior_W_logvar,
                enc_w1, enc_b1, enc_w2, enc_b2,
                gru_w_ih, gru_w_hh, gru_b_ih, gru_b_hh,
                dec_w, dec_b, em_w, em_b, ev_w, ev_b, T=SM1):
    f32 = np.float32
    X = np.asarray(X, f32)
    A = np.asarray(A, f32)
    W_eps = np.asarray(W_eps, f32)
    W_mu = np.asarray(W_mu, f32)
    W_logvar = np.asarray(W_logvar, f32)
    sig = np.exp(0.5 * W_logvar).astype(f32)
    w_hh = np.asarray(gru_w_hh, f32)
    w_ih = np.asarray(gru_w_ih, f32)
    b_ih = np.asarray(gru_b_ih, f32)
    b_hh = np.asarray(gru_b_hh, f32)

    whh8 = np.ascontiguousarray(
        w_hh.T.reshape(KT, 128, MT3, 128).transpose(1, 0, 2, 3)
        .reshape(128, KT * MT3 * 128)).astype(NP_F8)
    wih_aug = np.concatenate([
        w_ih.T,
        np.concatenate([(b_ih + b_hh)[:2 * H], b_ih[2 * H:]])[None, :],
    ], axis=0)
    wih8 = np.ascontiguousarray(wih_aug).astype(NP_F8)
    bhhn8 = np.ascontiguousarray(b_hh[2 * H:][None, :]).astype(NP_F8)
    ones8 = np.ones((1, BC), NP_F8)

    ew1 = np.ascontiguousarray(np.asarray(enc_w1, f32).T).astype(np.float16)
    eb1 = np.ascontiguousarray(np.asarray(enc_b1, f32).reshape(KT, 128).T)
    ew2l = np.ascontiguousarray(
        np.asarray(enc_w2, f32).T.reshape(KT, 128, KT, 128)
        .transpose(1, 0, 2, 3).reshape(128, KT * H)).astype(np.float16)
    eb2 = np.ascontiguousarray(np.asarray(enc_b2, f32).reshape(KT, 128).T)
    dwl = np.ascontiguousarray(
        np.asarray(dec_w, f32).T.reshape(KT, 128, KT, 128)
        .transpose(1, 0, 2, 3).reshape(128, KT * H)).astype(np.float16)
    db = np.ascontiguousarray(np.asarray(dec_b, f32).reshape(KT, 128).T)
    emwl = np.ascontiguousarray(
        np.asarray(em_w, f32).T.reshape(KT, 128, OUT)
        .transpose(1, 0, 2).reshape(128, KT * OUT)).astype(np.float16)
    evwl = np.ascontiguousarray(
        np.asarray(ev_w, f32).T.reshape(KT, 128, OUT)
        .transpose(1, 0, 2).reshape(128, KT * OUT)).astype(np.float16)
    evb = np.asarray(ev_b, f32).reshape(OUT, 1)
    em_b = np.asarray(em_b, f32)
    ident = np.eye(128, dtype=f32)

    shared = dict(
        wmu32=np.ascontiguousarray(W_mu),
        sig32=np.ascontiguousarray(sig),
        whh8=whh8, wih8=wih8, bhhn8=bhhn8, ones8=ones8,
        ew1=ew1, eb1=eb1, ew2l=ew2l, eb2=eb2,
        dwl=dwl, db=db, emwl=emwl, evwl=evwl, evb=evb,
        ident=ident,
        kl_lv=np.ascontiguousarray(W_logvar),
        kl_pm=np.ascontiguousarray(np.asarray(prior_W_mu, f32)),
        kl_pl=np.ascontiguousarray(np.asarray(prior_W_logvar, f32)),
    )

    in_maps = []
    for c in range(NC):
        bs = c * BC
        m = dict(shared)
        m["eps"] = np.ascontiguousarray(W_eps[bs:bs + BC])
        a_tr = A[:, bs:bs + BC, :].transpose(2, 0, 1)  # [8, T, BC]
        a_aug = np.concatenate(
            [a_tr, np.ones((1, SM1, BC), f32)], axis=0)
        m["a_t"] = np.ascontiguousarray(a_aug).astype(NP_F8)
        m["x0"] = np.ascontiguousarray(X[0, bs:bs + BC, :].T).astype(np.float16)
        m["xt"] = np.ascontiguousarray(
            (X[1:, bs:bs + BC, :] - em_b[None, None, :])
            .transpose(2, 0, 1).reshape(OUT, SM1 * BC))
        in_maps.append(m)
    return in_maps


_NC_CACHE = {}


def kernel(X, A, W_eps, W_mu, W_logvar, prior_W_mu, prior_W_logvar,
           enc_w1, enc_b1, enc_w2, enc_b2,
           gru_w_ih, gru_w_hh, gru_b_ih, gru_b_hh,
           dec_w, dec_b, em_w, em_b, ev_w, ev_b, N):
    in_maps = prep_inputs(X, A, W_eps, W_mu, W_logvar, prior_W_mu,
                          prior_W_logvar, enc_w1, enc_b1, enc_w2, enc_b2,
                          gru_w_ih, gru_w_hh, gru_b_ih, gru_b_hh,
                          dec_w, dec_b, em_w, em_b, ev_w, ev_b)
    if SM1 not in _NC_CACHE:
        _NC_CACHE[SM1] = build_nc(SM1)
    nc = _NC_CACHE[SM1]
    res = run_bass_kernel_spmd(nc, in_maps, core_ids=list(range(NC)))
    outs = [r["out"] for r in res.results]
    LL = float(sum(o[0, 0] for o in outs))
    KL = float(outs[0][0, 1])
    LL_n = LL / B
    KL_n = KL / (SM1 * B)
    return np.array([LL_n - KL_n, LL_n, KL_n], dtype=np.float32)


# revision 21
# speedup vs baseline: 1.1219x; 1.1219x over previous
"""Bayesian RNN (BRNN) Trainium2 kernel.

Data-parallel over batch: each of 8 NeuronCores handles 16 of the 128
batch samples.  The per-sample sampled weight W[b] = W_mu + sigma*eps[b]
(1024x1025 each) is built on-device, transposed, quantized to fp8-e4m3
and kept fully resident in SBUF (16.8 MB/core), so the 255-step
recurrence runs with zero HBM traffic for the weights.  The recurrent
loop computes the GRU cell + per-sample matvec on the tensor engine
(fp8 stationaries, fp32 PSUM accumulation); the decoder / emission /
log-likelihood stage is deferred and done as dense matmuls after the
scan.  KL is computed in fp32 on every core (identical values); the
host sums the 8 per-core LL partials.

Numerics validated against the fp32 reference scheme: fp8 weights with
fp16 master activations give ~6e-4 relative error on the final scalars
(tolerance 2e-2).
"""

import numpy as np
import ml_dtypes
import orjson

import concourse.bass as bass
import concourse.mybir as mybir
import concourse.tile as tile
import concourse.bass2jax as bass2jax
from concourse.bass_utils import (
    run_bass_kernel_spmd,
    compile_bir_kernel as _orig_compile_bir_kernel,
)

H = 1024
OUT = 32
ACTD = 8
B = 128
SEQ = 256
SM1 = SEQ - 1          # 255 recurrent steps
NC = 8                 # cores
BC = B // NC           # 16 batch samples per core
KT = H // 128          # 8 h-tiles
MT3 = 3 * H // 128     # 24 gru output tiles
LOG2PI = float(np.log(2.0 * np.pi))

F32 = mybir.dt.float32
F16 = mybir.dt.float16
F8 = mybir.dt.float8e4
AF = mybir.ActivationFunctionType
ALU = mybir.AluOpType
PSUM = bass.MemorySpace.PSUM

NP_F8 = ml_dtypes.float8_e4m3


# ---------------------------------------------------------------------------
# walrus on this container caps sync-waits at 1 per instruction; tile's
# kernel-tail drain (and some barriers) carry several.  Split extra waits
# onto same-engine NoOps ahead of the instruction (order-preserving, so
# semantically identical).
def _split_waits(bir):
    n = 0
    for f in bir["functions"]:
        for blk in f["blocks"]:
            out_insts = []
            for ins in blk["instructions"]:
                si = ins.get("sync_info")
                if si and len(si.get("on_wait", [])) > 1:
                    waits = si["on_wait"]
                    for w in waits[:-1]:
                        n += 1
                        out_insts.append({
                            "engine": ins["engine"],
                            "ins": [], "outs": [],
                            "name": f"I-waitsplit-{n}",
                            "opcode": "NoOp",
                            "sync_info": {"on_update": [], "on_wait": [w]},
                        })
                    si["on_wait"] = waits[-1:]
                out_insts.append(ins)
            blk["instructions"] = out_insts
    return bir


def _patched_compile(bir_json, tmpdir, neff_name="file.neff"):
    bir = _split_waits(orjson.loads(bir_json))
    return _orig_compile_bir_kernel(orjson.dumps(bir), tmpdir, neff_name)


def _enable_ldw_opt():
    import concourse.bass_utils as bu
    orig = bu.bir_verify_and_optimise

    def patched(tmpdir, inp="bir.json", outp="file.neff", arch=None, *,
                dve_root=None):
        import subprocess
        real_run = bu.run_command

        def run_hook(argv, **kw):
            argv = [a.replace("--enable-ldw-opt=false",
                              "--enable-ldw-opt=true") for a in argv]
            return real_run(argv, **kw)
        bu.run_command = run_hook
        try:
            return orig(tmpdir, inp, outp, arch, dve_root=dve_root)
        finally:
            bu.run_command = real_run
    bu.bir_verify_and_optimise = patched


bass2jax.compile_bir_kernel = _patched_compile


# ---------------------------------------------------------------------------
def build_nc(T=SM1, no_bmm=False, no_gru=False, reps=1):
    nc = bass.Bass("TRN2", target_bir_lowering=False, debug=False,
                   num_devices=NC)

    def inp(name, shape, dt):
        return nc.declare_dram_parameter(name, list(shape), dt, isOutput=False)

    eps_in = inp("eps", [BC, H, H + 1], F32)
    at_in = inp("a_t", [ACTD + 1, SM1, BC], F8)
    x0_in = inp("x0", [OUT, BC], F16)
    xt_in = inp("xt", [OUT, SM1 * BC], F32)
    wmu_in = inp("wmu32", [H, H + 1], F32)
    sig_in = inp("sig32", [H, H + 1], F32)
    whh_in = inp("whh8", [128, KT * MT3 * 128], F8)
    wih_in = inp("wih8", [ACTD + 1, 3 * H], F8)
    bhhn_in = inp("bhhn8", [1, H], F8)
    ones_in = inp("ones8", [1, BC], F8)
    ew1_in = inp("ew1", [OUT, H], F16)
    eb1_in = inp("eb1", [128, KT], F32)
    ew2_in = inp("ew2l", [128, KT * H], F16)
    eb2_in = inp("eb2", [128, KT], F32)
    dw_in = inp("dwl", [128, KT * H], F16)
    db_in = inp("db", [128, KT], F32)
    emw_in = inp("emwl", [128, KT * OUT], F16)
    evw_in = inp("evwl", [128, KT * OUT], F16)
    evb_in = inp("evb", [OUT, 1], F32)
    id_in = inp("ident", [128, 128], F32)
    kllv_in = inp("kl_lv", [H, H + 1], F32)
    klpm_in = inp("kl_pm", [H, H + 1], F32)
    klpl_in = inp("kl_pl", [H, H + 1], F32)
    out_ext = nc.declare_dram_parameter("out", [1, 2], F32, isOutput=True)

    nh_dram = nc.dram_tensor("nh_all", [SM1, 128 * 128], F16)

    with tile.TileContext(nc) as tc:
        _rep = tc.For_i(0, reps, 1) if reps > 1 else None
        if _rep is not None:
            _rep.__enter__()
        with tc.tile_pool(name="main", bufs=1) as mp:
            # ---- persistent (through the recurrent loop) tiles ----
            wt_sb = mp.tile([128, BC * KT * H], F8)      # W_T resident
            whh_sb = mp.tile([128, KT * MT3 * 128], F8)
            wih_sb = mp.tile([ACTD + 1, 3 * H], F8)
            a_sb = mp.tile([ACTD + 1, SM1, BC], F8)
            bhhn_sb = mp.tile([1, H], F8)
            ones_sb = mp.tile([1, BC], F8)
            bias_sb = mp.tile([128, 128], F32)           # bmm bias, (it,b)
            h_sb = mp.tile([128, 128], F16)              # carry, (kt,b)
            ident_sb = mp.tile([128, 128], F32)

            nc.sync.dma_start(out=whh_sb[:, :], in_=whh_in.ap()[:, :])
            nc.sync.dma_start(out=wih_sb[:, :], in_=wih_in.ap()[:, :])
            nc.sync.dma_start(out=a_sb[:, :, :], in_=at_in.ap()[:, :, :])
            nc.sync.dma_start(out=bhhn_sb[:, :], in_=bhhn_in.ap()[:, :])
            nc.sync.dma_start(out=ones_sb[:, :], in_=ones_in.ap()[:, :])
            nc.sync.dma_start(out=ident_sb[:, :], in_=id_in.ap()[:, :])

            # ---- initial encoder: h0 = tanh(relu(x0@w1.T+b1)@w2.T+b2) ----
            with tc.tile_pool(name="enc", bufs=1) as ep, \
                 tc.tile_pool(name="encps", bufs=2, space=PSUM) as eps_ps:
                ew1_sb = ep.tile([OUT, H], F16)
                eb1_sb = ep.tile([128, KT], F32)
                ew2_sb = ep.tile([128, KT * H], F16)
                eb2_sb = ep.tile([128, KT], F32)
                x0_sb = ep.tile([OUT, BC], F16)
                e1_sb = ep.tile([128, KT * BC], F16)
                nc.sync.dma_start(out=ew1_sb[:, :], in_=ew1_in.ap()[:, :])
                nc.sync.dma_start(out=eb1_sb[:, :], in_=eb1_in.ap()[:, :])
                nc.sync.dma_start(out=ew2_sb[:, :], in_=ew2_in.ap()[:, :])
                nc.sync.dma_start(out=eb2_sb[:, :], in_=eb2_in.ap()[:, :])
                nc.sync.dma_start(out=x0_sb[:, :], in_=x0_in.ap()[:, :])
                for mt in range(KT):
                    ps = eps_ps.tile([128, BC], F32, tag="encp")
                    nc.tensor.matmul(ps[:, :],
                                     ew1_sb[:, mt * 128:(mt + 1) * 128],
                                     x0_sb[:, :], start=True, stop=True)
                    nc.scalar.activation(e1_sb[:, mt * BC:(mt + 1) * BC],
                                         ps[:, :], AF.Relu,
                                         bias=eb1_sb[:, mt:mt + 1])
                for mt in range(KT):
                    ps = eps_ps.tile([128, BC], F32, tag="encp")
                    for kt in range(KT):
                        nc.tensor.matmul(
                            ps[:, :],
                            ew2_sb[:, kt * H + mt * 128: kt * H + (mt + 1) * 128],
                            e1_sb[:, kt * BC:(kt + 1) * BC],
                            start=(kt == 0), stop=(kt == KT - 1))
                    nc.scalar.activation(h_sb[:, mt * BC:(mt + 1) * BC],
                                         ps[:, :], AF.Tanh,
                                         bias=eb2_sb[:, mt:mt + 1])

            # ---- W transform: W_T[b] = (W_mu + sigma*eps[b]).T -> fp8 ----
            with tc.tile_pool(name="xf", bufs=1) as xp, \
                 tc.tile_pool(name="xfps", bufs=4, space=PSUM) as xps:
                for it in range(KT):
                    mu_t = xp.tile([128, H + 1], F32, tag="mu")
                    sg_t = xp.tile([128, H + 1], F32, tag="sg")
                    nc.sync.dma_start(
                        out=mu_t[:, :],
                        in_=wmu_in.ap()[it * 128:(it + 1) * 128, :])
                    nc.sync.dma_start(
                        out=sg_t[:, :],
                        in_=sig_in.ap()[it * 128:(it + 1) * 128, :])
                    for b in range(BC):
                        et = xp.tile([128, H + 1], F32, tag="eps")
                        nc.sync.dma_start(
                            out=et[:, :],
                            in_=eps_in.ap()[b, it * 128:(it + 1) * 128, :])
                        w32 = xp.tile([128, H], F32, tag="w32")
                        nc.vector.tensor_tensor(w32[:, :], et[:, :H],
                                                sg_t[:, :H], op=ALU.mult)
                        nc.vector.tensor_tensor(w32[:, :], w32[:, :],
                                                mu_t[:, :H], op=ALU.add)
                        nc.vector.scalar_tensor_tensor(
                            bias_sb[:, it * BC + b: it * BC + b + 1],
                            et[:, H:H + 1], sg_t[:, H:H + 1],
                            mu_t[:, H:H + 1], op0=ALU.mult, op1=ALU.add)
                        for jt in range(KT):
                            tp = xps.tile([128, 128], F32, tag="tp")
                            nc.tensor.transpose(
                                tp[:, :], w32[:, jt * 128:(jt + 1) * 128],
                                ident_sb[:, :])
                            col = (b * KT + jt) * H + it * 128
                            nc.scalar.copy(wt_sb[:, col:col + 128], tp[:, :])

            # ---- recurrent loop (3-step unrolled) ----
            UNR = 3 if T % 3 == 0 else 1
            a_re = a_sb[:, :, :].rearrange("k (g u) b -> k g u b", u=UNR)
            nh_re_st = nh_dram.ap().rearrange("(g u) f -> g u f", u=UNR)
            with tc.tile_pool(name="lp", bufs=2) as lp, \
                 tc.tile_pool(name="lps", bufs=2, space=PSUM) as lps:
                with tc.For_i(0, T // UNR, 1,
                              hint_engines=(mybir.EngineType.PE,)) as i:
                  for uu in range(UNR):
                    ghrz = lps.tile([128, 16 * BC], F32, tag="ghrz")
                    ghn = lps.tile([128, KT * BC], F32, tag="ghn")
                    gin = lps.tile([128, KT * BC], F32, tag="gin")
                    a_t0 = lp.tile([ACTD + 1, BC], F8, tag="a_t")
                    nc.vector.tensor_copy(a_t0[:, :],
                                          a_re[:, bass.ds(i, 1), uu, :])
                    a_t = a_t0[:, :]
                    nkt = 1 if no_gru else KT
                    for mt in range(MT3):
                        dst = (ghrz[:, mt * BC:(mt + 1) * BC] if mt < 16
                               else ghn[:, (mt - 16) * BC:(mt - 15) * BC])
                        for kt in range(nkt):
                            nc.tensor.matmul(
                                dst,
                                whh_sb[:, (kt * MT3 + mt) * 128:
                                       (kt * MT3 + mt + 1) * 128],
                                h_sb[:, kt * BC:(kt + 1) * BC],
                                start=(kt == 0), stop=False)
                        if mt < 16:
                            # gi + (b_ih+b_hh) ride the K=9 augmented row
                            nc.tensor.matmul(
                                dst,
                                wih_sb[:, mt * 128:(mt + 1) * 128],
                                a_t, start=False, stop=True)
                        else:
                            # n-gate hh side: + b_hh_n via K=1 ones matmul
                            nc.tensor.matmul(
                                dst,
                                bhhn_sb[:, (mt - 16) * 128:(mt - 15) * 128],
                                ones_sb[:, :], start=False, stop=True)
                    for mt in range(KT):
                        # gi_n + b_ih_n (augmented row)
                        nc.tensor.matmul(
                            gin[:, mt * BC:(mt + 1) * BC],
                            wih_sb[:, (16 + mt) * 128:(17 + mt) * 128],
                            a_t, start=True, stop=True)

                    # sigmoid(x) = 0.5*tanh(x/2)+0.5 -- keeps ACT on the
                    # tanh table all loop long (no table reloads)
                    rz_sb = lp.tile([128, 256], F32, tag="rz")
                    nc.scalar.activation(rz_sb[:, :], ghrz[:, :], AF.Tanh,
                                         scale=0.5)
                    rz2 = lp.tile([128, 256], F32, tag="rz2")
                    nc.vector.tensor_scalar(rz2[:, :], rz_sb[:, :], 0.5, 0.5,
                                            ALU.mult, ALU.add)
                    t1 = lp.tile([128, 128], F32, tag="t1")
                    nc.vector.tensor_tensor(t1[:, :], ghn[:, :],
                                            rz2[:, 0:128], op=ALU.mult)
                    t2 = lp.tile([128, 128], F32, tag="t2")
                    nc.vector.tensor_tensor(t2[:, :], gin[:, :], t1[:, :],
                                            op=ALU.add)
                    n_sb = lp.tile([128, 128], F16, tag="n")
                    nc.scalar.activation(n_sb[:, :], t2[:, :], AF.Tanh)
                    d_sb = lp.tile([128, 128], F32, tag="d")
                    nc.vector.tensor_tensor(d_sb[:, :], h_sb[:, :],
                                            n_sb[:, :], op=ALU.subtract)
                    zd_sb = lp.tile([128, 128], F32, tag="zd")
                    nc.vector.tensor_tensor(zd_sb[:, :], rz2[:, 128:256],
                                            d_sb[:, :], op=ALU.mult)
                    hc_sb = lp.tile([128, 128], F16, tag="hc")
                    nc.vector.tensor_tensor(hc_sb[:, :], n_sb[:, :],
                                            zd_sb[:, :], op=ALU.add)

                    g_ps = lps.tile([128, 128], F32, tag="g")
                    if not no_bmm:
                        for b in range(BC):
                            for it in range(KT):
                                col = it * BC + b
                                for jt in range(KT):
                                    w0 = (b * KT + jt) * H + it * 128
                                    nc.tensor.matmul(
                                        g_ps[:, col:col + 1],
                                        wt_sb[:, w0:w0 + 128],
                                        hc_sb[:, jt * BC + b:jt * BC + b + 1],
                                        start=(jt == 0), stop=(jt == KT - 1))
                    else:
                        nc.tensor.matmul(g_ps[:, :], wt_sb[:, 0:128],
                                         hc_sb[:, :], start=True, stop=True)
                    g2 = lp.tile([128, 128], F32, tag="g2")
                    nc.vector.tensor_tensor(g2[:, :], g_ps[:, :],
                                            bias_sb[:, :], op=ALU.add)
                    nc.scalar.activation(h_sb[:, :], g2[:, :], AF.Tanh)
                    nc.sync.dma_start(out=nh_re_st[bass.ds(i, 1), uu, :],
                                      in_=h_sb[:, :])

        # ---- post pass: decoder, emission, log-likelihood, KL ----
        ROWS = T * BC
        NCH = (ROWS + 509) // 510
        with tc.tile_pool(name="post", bufs=1) as pp, \
             tc.tile_pool(name="postps", bufs=2, space=PSUM) as pps:
            dw_sb = pp.tile([128, KT * H], F16)
            db_sb = pp.tile([128, KT], F32)
            emw_sb = pp.tile([128, KT * OUT], F16)
            evw_sb = pp.tile([128, KT * OUT], F16)
            evb_sb = pp.tile([OUT, 1], F32)
            xt_sb = pp.tile([OUT, SM1 * BC], F32)
            acc_q = pp.tile([OUT, NCH], F32)
            acc_lv = pp.tile([OUT, NCH], F32)
            out_sb = pp.tile([1, 2], F32)
            nc.sync.dma_start(out=dw_sb[:, :], in_=dw_in.ap()[:, :])
            nc.sync.dma_start(out=db_sb[:, :], in_=db_in.ap()[:, :])
            nc.sync.dma_start(out=emw_sb[:, :], in_=emw_in.ap()[:, :])
            nc.sync.dma_start(out=evw_sb[:, :], in_=evw_in.ap()[:, :])
            nc.sync.dma_start(out=evb_sb[:, :], in_=evb_in.ap()[:, :])
            nc.sync.dma_start(out=xt_sb[:, :T * BC], in_=xt_in.ap()[:, :T * BC])

            nh_t = []
            nh_re = nh_dram.ap().rearrange("t (p kt b) -> p kt t b",
                                           p=128, kt=KT, b=BC)
            for kt in range(KT):
                t_ = pp.tile([128, ROWS], F16, tag=f"nh{kt}")
                nc.sync.dma_start(out=t_[:, :],
                                  in_=nh_re[:, kt, :T, :])
                nh_t.append(t_)

            p2_ctx = tc.tile_pool(name="post2", bufs=2)
            p2 = p2_ctx.__enter__()
            for rc in range(NCH):
                r0 = rc * 510
                rn = min(510, ROWS - r0)
                dec_t = []
                for mt in range(KT):
                    ps = pps.tile([128, 510], F32, tag="decp")
                    for kt in range(KT):
                        nc.tensor.matmul(
                            ps[:, :rn],
                            dw_sb[:, kt * H + mt * 128: kt * H + (mt + 1) * 128],
                            nh_t[kt][:, r0:r0 + rn],
                            start=(kt == 0), stop=(kt == KT - 1))
                    dt_ = p2.tile([128, 510], F16, tag=f"dec{mt}")
                    nc.scalar.activation(dt_[:, :rn], ps[:, :rn], AF.Relu,
                                         bias=db_sb[:, mt:mt + 1])
                    dec_t.append(dt_)
                m_ps = pps.tile([OUT, 510], F32, tag="mp")
                v_ps = pps.tile([OUT, 510], F32, tag="vp")
                for kt in range(KT):
                    nc.tensor.matmul(m_ps[:, :rn],
                                     emw_sb[:, kt * OUT:(kt + 1) * OUT],
                                     dec_t[kt][:, :rn],
                                     start=(kt == 0), stop=(kt == KT - 1))
                for kt in range(KT):
                    nc.tensor.matmul(v_ps[:, :rn],
                                     evw_sb[:, kt * OUT:(kt + 1) * OUT],
                                     dec_t[kt][:, :rn],
                                     start=(kt == 0), stop=(kt == KT - 1))
                # e = exp(-(v + evb)) ; d = xt' - m ; q = d*d*e
                e_sb = p2.tile([OUT, 510], F32, tag="e")
                nc.scalar.activation(e_sb[:, :rn], v_ps[:, :rn], AF.Exp,
                                     bias=evb_sb[:, 0:1], scale=-1.0)
                d_sb2 = p2.tile([OUT, 510], F32, tag="dd")
                nc.vector.tensor_tensor(d_sb2[:, :rn], xt_sb[:, r0:r0 + rn],
                                        m_ps[:, :rn], op=ALU.subtract)
                d2_sb = p2.tile([OUT, 510], F32, tag="d2")
                nc.vector.tensor_tensor(d2_sb[:, :rn], d_sb2[:, :rn],
                                        d_sb2[:, :rn], op=ALU.mult)
                q_sb = p2.tile([OUT, 510], F32, tag="q")
                nc.vector.scalar_tensor_tensor(
                    q_sb[:, :rn], d2_sb[:, :rn], 1.0, e_sb[:, :rn],
                    op0=ALU.mult, op1=ALU.mult,
                    accum_out=acc_q[:, rc:rc + 1])
                nc.vector.tensor_reduce(acc_lv[:, rc:rc + 1], v_ps[:, :rn],
                                        axis=mybir.AxisListType.X, op=ALU.add)

            p2_ctx.__exit__(None, None, None)

            # KL on every core (identical) in fp32
            acc_kl = pp.tile([128, KT], F32)
            p2_ctx = tc.tile_pool(name="klp", bufs=1)
            p2 = p2_ctx.__enter__()
            for it in range(KT):
                lv_t = p2.tile([128, H + 1], F32, tag="klv")
                pm_t = p2.tile([128, H + 1], F32, tag="kpm")
                pl_t = p2.tile([128, H + 1], F32, tag="kpl")
                mu_t = p2.tile([128, H + 1], F32, tag="kmu")
                sl = slice(it * 128, (it + 1) * 128)
                nc.sync.dma_start(out=lv_t[:, :], in_=kllv_in.ap()[sl, :])
                nc.sync.dma_start(out=pm_t[:, :], in_=klpm_in.ap()[sl, :])
                nc.sync.dma_start(out=pl_t[:, :], in_=klpl_in.ap()[sl, :])
                nc.sync.dma_start(out=mu_t[:, :], in_=wmu_in.ap()[sl, :])
                eq = p2.tile([128, H + 1], F32, tag="keq")
                nc.scalar.activation(eq[:, :], lv_t[:, :], AF.Exp)
                ep = p2.tile([128, H + 1], F32, tag="kep")
                nc.scalar.activation(ep[:, :], pl_t[:, :], AF.Exp, scale=-1.0)
                dmu = p2.tile([128, H + 1], F32, tag="kdm")
                nc.vector.tensor_tensor(dmu[:, :], mu_t[:, :], pm_t[:, :],
                                        op=ALU.subtract)
                d2m = p2.tile([128, H + 1], F32, tag="kd2")
                nc.vector.tensor_tensor(d2m[:, :], dmu[:, :], dmu[:, :],
                                        op=ALU.mult)
                s_ = p2.tile([128, H + 1], F32, tag="ks")
                nc.vector.tensor_tensor(s_[:, :], eq[:, :], d2m[:, :],
                                        op=ALU.add)
                t1_ = p2.tile([128, H + 1], F32, tag="kt1")
                nc.vector.tensor_tensor(t1_[:, :], s_[:, :], ep[:, :],
                                        op=ALU.mult)
                v1_ = p2.tile([128, H + 1], F32, tag="kv1")
                nc.vector.tensor_tensor(v1_[:, :], pl_t[:, :], lv_t[:, :],
                                        op=ALU.subtract)
                w_ = p2.tile([128, H + 1], F32, tag="kw")
                nc.vector.tensor_tensor(w_[:, :], v1_[:, :], t1_[:, :],
                                        op=ALU.add)
                kls = p2.tile([128, H + 1], F32, tag="kls")
                nc.vector.scalar_tensor_tensor(
                    kls[:, :], w_[:, :], -1.0, w_[:, :],
                    op0=ALU.add, op1=ALU.bypass,
                    accum_out=acc_kl[:, it:it + 1])

            p2_ctx.__exit__(None, None, None)

            # reductions to scalars
            s1a = pp.tile([OUT, 1], F32)
            nc.vector.tensor_reduce(s1a[:, :], acc_q[:, :],
                                    axis=mybir.AxisListType.X, op=ALU.add)
            s2a = pp.tile([OUT, 1], F32)
            nc.vector.tensor_reduce(s2a[:, :], acc_lv[:, :],
                                    axis=mybir.AxisListType.X, op=ALU.add)
            ka = pp.tile([128, 1], F32)
            nc.vector.tensor_reduce(ka[:, :], acc_kl[:, :],
                                    axis=mybir.AxisListType.X, op=ALU.add)
            s1 = pp.tile([1, 1], F32)
            nc.gpsimd.tensor_reduce(s1[:, :], s1a[:, :],
                                    axis=mybir.AxisListType.C, op=ALU.add)
            s2 = pp.tile([1, 1], F32)
            nc.gpsimd.tensor_reduce(s2[:, :], s2a[:, :],
                                    axis=mybir.AxisListType.C, op=ALU.add)
            sb_ = pp.tile([1, 1], F32)
            nc.gpsimd.tensor_reduce(sb_[:, :], evb_sb[:, :],
                                    axis=mybir.AxisListType.C, op=ALU.add)
            k1 = pp.tile([1, 1], F32)
            nc.gpsimd.tensor_reduce(k1[:, :], ka[:, :],
                                    axis=mybir.AxisListType.C, op=ALU.add)
            # LL = -0.5*(S1 + S2 + ROWS*SB + LOG2PI*NE)
            ne = float(ROWS * OUT)
            u1 = pp.tile([1, 1], F32)
            nc.vector.tensor_tensor(u1[:, :], s1[:, :], s2[:, :], op=ALU.add)
            u2 = pp.tile([1, 1], F32)
            nc.vector.scalar_tensor_tensor(u2[:, :], sb_[:, :], float(ROWS),
                                           u1[:, :], op0=ALU.mult, op1=ALU.add)
            u3 = pp.tile([1, 1], F32)
            nc.vector.scalar_tensor_tensor(u3[:, :], u2[:, :],
                                           LOG2PI * ne, u2[:, :],
                                           op0=ALU.add, op1=ALU.bypass)
            nc.vector.tensor_scalar_mul(out_sb[:, 0:1], u3[:, :], -0.5)
            nc.vector.tensor_scalar_mul(out_sb[:, 1:2], k1[:, :], 0.5)
            nc.sync.dma_start(out=out_ext.ap()[:, :], in_=out_sb[:, :])
        if _rep is not None:
            _rep.__exit__(None, None, None)

    return nc


# ---------------------------------------------------------------------------
def prep_inputs(X, A, W_eps, W_mu, W_logvar, prior_W_mu, prior_W_logvar,
                enc_w1, enc_b1, enc_w2, enc_b2,
                gru_w_ih, gru_w_hh, gru_b_ih, gru_b_hh,
                dec_w, dec_b, em_w, em_b, ev_w, ev_b, T=SM1):
    f32 = np.float32
    X = np.asarray(X, f32)
    A = np.asarray(A, f32)
    W_eps = np.asarray(W_eps, f32)
    W_mu = np.asarray(W_mu, f32)
    W_logvar = np.asarray(W_logvar, f32)
    sig = np.exp(0.5 * W_logvar).astype(f32)
    w_hh = np.asarray(gru_w_hh, f32)
    w_ih = np.asarray(gru_w_ih, f32)
    b_ih = np.asarray(gru_b_ih, f32)
    b_hh = np.asarray(gru_b_hh, f32)

    whh8 = np.ascontiguousarray(
        w_hh.T.reshape(KT, 128, MT3, 128).transpose(1, 0, 2, 3)
        .reshape(128, KT * MT3 * 128)).astype(NP_F8)
    wih_aug = np.concatenate([
        w_ih.T,
        np.concatenate([(b_ih + b_hh)[:2 * H], b_ih[2 * H:]])[None, :],
    ], axis=0)
    wih8 = np.ascontiguousarray(wih_aug).astype(NP_F8)
    bhhn8 = np.ascontiguousarray(b_hh[2 * H:][None, :]).astype(NP_F8)
    ones8 = np.ones((1, BC), NP_F8)

    ew1 = np.ascontiguousarray(np.asarray(enc_w1, f32).T).astype(np.float16)
    eb1 = np.ascontiguousarray(np.asarray(enc_b1, f32).reshape(KT, 128).T)
    ew2l = np.ascontiguousarray(
        np.asarray(enc_w2, f32).T.reshape(KT, 128, KT, 128)
        .transpose(1, 0, 2, 3).reshape(128, KT * H)).astype(np.float16)
    eb2 = np.ascontiguousarray(np.asarray(enc_b2, f32).reshape(KT, 128).T)
    dwl = np.ascontiguousarray(
        np.asarray(dec_w, f32).T.reshape(KT, 128, KT, 128)
        .transpose(1, 0, 2, 3).reshape(128, KT * H)).astype(np.float16)
    db = np.ascontiguousarray(np.asarray(dec_b, f32).reshape(KT, 128).T)
    emwl = np.ascontiguousarray(
        np.asarray(em_w, f32).T.reshape(KT, 128, OUT)
        .transpose(1, 0, 2).reshape(128, KT * OUT)).astype(np.float16)
    evwl = np.ascontiguousarray(
        np.asarray(ev_w, f32).T.reshape(KT, 128, OUT)
        .transpose(1, 0, 2).reshape(128, KT * OUT)).astype(np.float16)
    evb = np.asarray(ev_b, f32).reshape(OUT, 1)
    em_b = np.asarray(em_b, f32)
    ident = np.eye(128, dtype=f32)

    shared = dict(
        wmu32=np.ascontiguousarray(W_mu),
        sig32=np.ascontiguousarray(sig),
        whh8=whh8, wih8=wih8, bhhn8=bhhn8, ones8=ones8,
        ew1=ew1, eb1=eb1, ew2l=ew2l, eb2=eb2,
        dwl=dwl, db=db, emwl=emwl, evwl=evwl, evb=evb,
        ident=ident,
        kl_lv=np.ascontiguousarray(W_logvar),
        kl_pm=np.ascontiguousarray(np.asarray(prior_W_mu, f32)),
        kl_pl=np.ascontiguousarray(np.asarray(prior_W_logvar, f32)),
    )

    in_maps = []
    for c in range(NC):
        bs = c * BC
        m = dict(shared)
        m["eps"] = np.ascontiguousarray(W_eps[bs:bs + BC])
        a_tr = A[:, bs:bs + BC, :].transpose(2, 0, 1)  # [8, T, BC]
        a_aug = np.concatenate(
            [a_tr, np.ones((1, SM1, BC), f32)], axis=0)
        m["a_t"] = np.ascontiguousarray(a_aug).astype(NP_F8)
        m["x0"] = np.ascontiguousarray(X[0, bs:bs + BC, :].T).astype(np.float16)
        m["xt"] = np.ascontiguousarray(
            (X[1:, bs:bs + BC, :] - em_b[None, None, :])
            .transpose(2, 0, 1).reshape(OUT, SM1 * BC))
        in_maps.append(m)
    return in_maps


_NC_CACHE = {}


def kernel(X, A, W_eps, W_mu, W_logvar, prior_W_mu, prior_W_logvar,
           enc_w1, enc_b1, enc_w2, enc_b2,
           gru_w_ih, gru_w_hh, gru_b_ih, gru_b_hh,
           dec_w, dec_b, em_w, em_b, ev_w, ev_b, N):
    in_maps = prep_inputs(X, A, W_eps, W_mu, W_logvar, prior_W_mu,
                          prior_W_logvar, enc_w1, enc_b1, enc_w2, enc_b2,
                          gru_w_ih, gru_w_hh, gru_b_ih, gru_b_hh,
                          dec_w, dec_b, em_w, em_b, ev_w, ev_b)
    if SM1 not in _NC_CACHE:
        _NC_CACHE[SM1] = build_nc(SM1)
    nc = _NC_CACHE[SM1]
    res = run_bass_kernel_spmd(nc, in_maps, core_ids=list(range(NC)))
    outs = [r["out"] for r in res.results]
    LL = float(sum(o[0, 0] for o in outs))
    KL = float(outs[0][0, 1])
    LL_n = LL / B
    KL_n = KL / (SM1 * B)
    return np.array([LL_n - KL_n, LL_n, KL_n], dtype=np.float32)


# revision 22
# speedup vs baseline: 1.2372x; 1.1028x over previous
"""Bayesian RNN (BRNN) Trainium2 kernel.

Data-parallel over batch: each of 8 NeuronCores handles 16 of the 128
batch samples.  The per-sample sampled weight W[b] = W_mu + sigma*eps[b]
(1024x1025 each) is built on-device, transposed, quantized to fp8-e4m3
and kept fully resident in SBUF (16.8 MB/core), so the 255-step
recurrence runs with zero HBM traffic for the weights.  The recurrent
loop computes the GRU cell + per-sample matvec on the tensor engine
(fp8 stationaries, fp32 PSUM accumulation); the decoder / emission /
log-likelihood stage is deferred and done as dense matmuls after the
scan.  KL is computed in fp32 on every core (identical values); the
host sums the 8 per-core LL partials.

Numerics validated against the fp32 reference scheme: fp8 weights with
fp16 master activations give ~6e-4 relative error on the final scalars
(tolerance 2e-2).
"""

import numpy as np
import ml_dtypes
import orjson

import concourse.bass as bass
import concourse.mybir as mybir
import concourse.tile as tile
import concourse.bass2jax as bass2jax
from concourse.bass_utils import (
    run_bass_kernel_spmd,
    compile_bir_kernel as _orig_compile_bir_kernel,
)

H = 1024
OUT = 32
ACTD = 8
B = 128
SEQ = 256
SM1 = SEQ - 1          # 255 recurrent steps
NC = 8                 # cores
BC = B // NC           # 16 batch samples per core
KT = H // 128          # 8 h-tiles
MT3 = 3 * H // 128     # 24 gru output tiles
LOG2PI = float(np.log(2.0 * np.pi))

F32 = mybir.dt.float32
F16 = mybir.dt.float16
F8 = mybir.dt.float8e4
AF = mybir.ActivationFunctionType
ALU = mybir.AluOpType
PSUM = bass.MemorySpace.PSUM

NP_F8 = ml_dtypes.float8_e4m3


# ---------------------------------------------------------------------------
# walrus on this container caps sync-waits at 1 per instruction; tile's
# kernel-tail drain (and some barriers) carry several.  Split extra waits
# onto same-engine NoOps ahead of the instruction (order-preserving, so
# semantically identical).
def _split_waits(bir):
    n = 0
    for f in bir["functions"]:
        for blk in f["blocks"]:
            out_insts = []
            for ins in blk["instructions"]:
                si = ins.get("sync_info")
                if si and len(si.get("on_wait", [])) > 1:
                    waits = si["on_wait"]
                    for w in waits[:-1]:
                        n += 1
                        out_insts.append({
                            "engine": ins["engine"],
                            "ins": [], "outs": [],
                            "name": f"I-waitsplit-{n}",
                            "opcode": "NoOp",
                            "sync_info": {"on_update": [], "on_wait": [w]},
                        })
                    si["on_wait"] = waits[-1:]
                out_insts.append(ins)
            blk["instructions"] = out_insts
    return bir


def _patched_compile(bir_json, tmpdir, neff_name="file.neff"):
    bir = _split_waits(orjson.loads(bir_json))
    return _orig_compile_bir_kernel(orjson.dumps(bir), tmpdir, neff_name)


def _enable_ldw_opt():
    import concourse.bass_utils as bu
    orig = bu.bir_verify_and_optimise

    def patched(tmpdir, inp="bir.json", outp="file.neff", arch=None, *,
                dve_root=None):
        import subprocess
        real_run = bu.run_command

        def run_hook(argv, **kw):
            argv = [a.replace("--enable-ldw-opt=false",
                              "--enable-ldw-opt=true") for a in argv]
            return real_run(argv, **kw)
        bu.run_command = run_hook
        try:
            return orig(tmpdir, inp, outp, arch, dve_root=dve_root)
        finally:
            bu.run_command = real_run
    bu.bir_verify_and_optimise = patched


bass2jax.compile_bir_kernel = _patched_compile


# ---------------------------------------------------------------------------
def build_nc(T=SM1, no_bmm=False, no_gru=False, reps=1):
    nc = bass.Bass("TRN2", target_bir_lowering=False, debug=False,
                   num_devices=NC)

    def inp(name, shape, dt):
        return nc.declare_dram_parameter(name, list(shape), dt, isOutput=False)

    eps_in = inp("eps", [BC, H, H + 1], F32)
    at_in = inp("a_t", [ACTD + 1, SM1, BC], F8)
    x0_in = inp("x0", [OUT, BC], F16)
    xt_in = inp("xt", [OUT, SM1 * BC], F32)
    wmu_in = inp("wmu32", [H, H + 1], F32)
    sig_in = inp("sig32", [H, H + 1], F32)
    whh_in = inp("whh8", [128, KT * MT3 * 128], F8)
    wih_in = inp("wih8", [ACTD + 1, 3 * H], F8)
    bhhn_in = inp("bhhn8", [1, H], F8)
    ones_in = inp("ones8", [1, BC], F8)
    ew1_in = inp("ew1", [OUT, H], F16)
    eb1_in = inp("eb1", [128, KT], F32)
    ew2_in = inp("ew2l", [128, KT * H], F16)
    eb2_in = inp("eb2", [128, KT], F32)
    dw_in = inp("dwl", [128, KT * H], F16)
    db_in = inp("db", [128, KT], F32)
    emw_in = inp("emwl", [128, KT * OUT], F16)
    evw_in = inp("evwl", [128, KT * OUT], F16)
    evb_in = inp("evb", [OUT, 1], F32)
    id_in = inp("ident", [128, 128], F32)
    kllv_in = inp("kl_lv", [H, H + 1], F32)
    klpm_in = inp("kl_pm", [H, H + 1], F32)
    klpl_in = inp("kl_pl", [H, H + 1], F32)
    out_ext = nc.declare_dram_parameter("out", [1, 2], F32, isOutput=True)

    nh_dram = nc.dram_tensor("nh_all", [SM1, 128 * 128], F16)

    with tile.TileContext(nc) as tc:
        _rep = tc.For_i(0, reps, 1) if reps > 1 else None
        if _rep is not None:
            _rep.__enter__()
        with tc.tile_pool(name="main", bufs=1) as mp:
            # ---- persistent (through the recurrent loop) tiles ----
            wt_sb = mp.tile([128, BC * KT * H], F8)      # W_T resident
            whh_sb = mp.tile([128, KT * MT3 * 128], F8)
            wih_sb = mp.tile([ACTD + 1, 3 * H], F8)
            a_sb = mp.tile([ACTD + 1, SM1, BC], F8)
            bhhn_sb = mp.tile([1, H], F8)
            ones_sb = mp.tile([1, BC], F8)
            bias_sb = mp.tile([128, 128], F32)           # bmm bias, (it,b)
            h_sb = mp.tile([128, 128], F16)              # carry, (kt,b)
            ident_sb = mp.tile([128, 128], F32)

            nc.sync.dma_start(out=whh_sb[:, :], in_=whh_in.ap()[:, :])
            nc.sync.dma_start(out=wih_sb[:, :], in_=wih_in.ap()[:, :])
            nc.sync.dma_start(out=a_sb[:, :, :], in_=at_in.ap()[:, :, :])
            nc.sync.dma_start(out=bhhn_sb[:, :], in_=bhhn_in.ap()[:, :])
            nc.sync.dma_start(out=ones_sb[:, :], in_=ones_in.ap()[:, :])
            nc.sync.dma_start(out=ident_sb[:, :], in_=id_in.ap()[:, :])

            # ---- initial encoder: h0 = tanh(relu(x0@w1.T+b1)@w2.T+b2) ----
            with tc.tile_pool(name="enc", bufs=1) as ep, \
                 tc.tile_pool(name="encps", bufs=2, space=PSUM) as eps_ps:
                ew1_sb = ep.tile([OUT, H], F16)
                eb1_sb = ep.tile([128, KT], F32)
                ew2_sb = ep.tile([128, KT * H], F16)
                eb2_sb = ep.tile([128, KT], F32)
                x0_sb = ep.tile([OUT, BC], F16)
                e1_sb = ep.tile([128, KT * BC], F16)
                nc.sync.dma_start(out=ew1_sb[:, :], in_=ew1_in.ap()[:, :])
                nc.sync.dma_start(out=eb1_sb[:, :], in_=eb1_in.ap()[:, :])
                nc.sync.dma_start(out=ew2_sb[:, :], in_=ew2_in.ap()[:, :])
                nc.sync.dma_start(out=eb2_sb[:, :], in_=eb2_in.ap()[:, :])
                nc.sync.dma_start(out=x0_sb[:, :], in_=x0_in.ap()[:, :])
                for mt in range(KT):
                    ps = eps_ps.tile([128, BC], F32, tag="encp")
                    nc.tensor.matmul(ps[:, :],
                                     ew1_sb[:, mt * 128:(mt + 1) * 128],
                                     x0_sb[:, :], start=True, stop=True)
                    nc.scalar.activation(e1_sb[:, mt * BC:(mt + 1) * BC],
                                         ps[:, :], AF.Relu,
                                         bias=eb1_sb[:, mt:mt + 1])
                for mt in range(KT):
                    ps = eps_ps.tile([128, BC], F32, tag="encp")
                    for kt in range(KT):
                        nc.tensor.matmul(
                            ps[:, :],
                            ew2_sb[:, kt * H + mt * 128: kt * H + (mt + 1) * 128],
                            e1_sb[:, kt * BC:(kt + 1) * BC],
                            start=(kt == 0), stop=(kt == KT - 1))
                    nc.scalar.activation(h_sb[:, mt * BC:(mt + 1) * BC],
                                         ps[:, :], AF.Tanh,
                                         bias=eb2_sb[:, mt:mt + 1])

            # ---- W transform: W_T[b] = (W_mu + sigma*eps[b]).T -> fp8 ----
            with tc.tile_pool(name="xf", bufs=1) as xp, \
                 tc.tile_pool(name="xfps", bufs=4, space=PSUM) as xps:
                for it in range(KT):
                    mu_t = xp.tile([128, H + 1], F32, tag="mu")
                    sg_t = xp.tile([128, H + 1], F32, tag="sg")
                    nc.sync.dma_start(
                        out=mu_t[:, :],
                        in_=wmu_in.ap()[it * 128:(it + 1) * 128, :])
                    nc.sync.dma_start(
                        out=sg_t[:, :],
                        in_=sig_in.ap()[it * 128:(it + 1) * 128, :])
                    for b in range(BC):
                        et = xp.tile([128, H + 1], F32, tag="eps")
                        nc.sync.dma_start(
                            out=et[:, :],
                            in_=eps_in.ap()[b, it * 128:(it + 1) * 128, :])
                        w32 = xp.tile([128, H], F32, tag="w32")
                        nc.vector.tensor_tensor(w32[:, :], et[:, :H],
                                                sg_t[:, :H], op=ALU.mult)
                        nc.vector.tensor_tensor(w32[:, :], w32[:, :],
                                                mu_t[:, :H], op=ALU.add)
                        nc.vector.scalar_tensor_tensor(
                            bias_sb[:, it * BC + b: it * BC + b + 1],
                            et[:, H:H + 1], sg_t[:, H:H + 1],
                            mu_t[:, H:H + 1], op0=ALU.mult, op1=ALU.add)
                        for jt in range(KT):
                            tp = xps.tile([128, 128], F32, tag="tp")
                            nc.tensor.transpose(
                                tp[:, :], w32[:, jt * 128:(jt + 1) * 128],
                                ident_sb[:, :])
                            col = (b * KT + jt) * H + it * 128
                            nc.scalar.copy(wt_sb[:, col:col + 128], tp[:, :])

            # ---- recurrent loop (3-step unrolled) ----
            UNR = 3 if T % 3 == 0 else 1
            a_re = a_sb[:, :, :].rearrange("k (g u) b -> k g u b", u=UNR)
            nh_re_st = nh_dram.ap().rearrange("(g u) f -> g u f", u=UNR)
            with tc.tile_pool(name="lp", bufs=2) as lp, \
                 tc.tile_pool(name="lps", bufs=2, space=PSUM) as lps:
                with tc.For_i(0, T // UNR, 1,
                              hint_engines=(mybir.EngineType.PE,)) as i:
                  for uu in range(UNR):
                    ghrz = lps.tile([128, 16 * BC], F32, tag="ghrz")
                    ghn = lps.tile([128, KT * BC], F32, tag="ghn")
                    gin = lps.tile([128, KT * BC], F32, tag="gin")
                    a_t0 = lp.tile([ACTD + 1, BC], F8, tag="a_t")
                    nc.vector.tensor_copy(a_t0[:, :],
                                          a_re[:, bass.ds(i, 1), uu, :])
                    a_t = a_t0[:, :]
                    for mt in range(KT):
                        # gi_n + b_ih_n (augmented row); only needs a_t, so
                        # issue before the gh matmuls to keep it off the
                        # t2-gate critical path
                        nc.tensor.matmul(
                            gin[:, mt * BC:(mt + 1) * BC],
                            wih_sb[:, (16 + mt) * 128:(17 + mt) * 128],
                            a_t, start=True, stop=True)
                    nkt = 1 if no_gru else KT
                    for mt in range(MT3):
                        dst = (ghrz[:, mt * BC:(mt + 1) * BC] if mt < 16
                               else ghn[:, (mt - 16) * BC:(mt - 15) * BC])
                        for kt in range(nkt):
                            nc.tensor.matmul(
                                dst,
                                whh_sb[:, (kt * MT3 + mt) * 128:
                                       (kt * MT3 + mt + 1) * 128],
                                h_sb[:, kt * BC:(kt + 1) * BC],
                                start=(kt == 0), stop=False)
                        if mt < 16:
                            # gi + (b_ih+b_hh) ride the K=9 augmented row
                            nc.tensor.matmul(
                                dst,
                                wih_sb[:, mt * 128:(mt + 1) * 128],
                                a_t, start=False, stop=True)
                        else:
                            # n-gate hh side: + b_hh_n via K=1 ones matmul
                            nc.tensor.matmul(
                                dst,
                                bhhn_sb[:, (mt - 16) * 128:(mt - 15) * 128],
                                ones_sb[:, :], start=False, stop=True)
                    # sigmoid(x) = 0.5*tanh(x/2)+0.5 -- keeps ACT on the
                    # tanh table all loop long (no table reloads)
                    rz_sb = lp.tile([128, 256], F32, tag="rz")
                    nc.scalar.activation(rz_sb[:, :], ghrz[:, :], AF.Tanh,
                                         scale=0.5)
                    rz2 = lp.tile([128, 256], F32, tag="rz2")
                    nc.vector.tensor_scalar(rz2[:, :], rz_sb[:, :], 0.5, 0.5,
                                            ALU.mult, ALU.add)
                    t1 = lp.tile([128, 128], F32, tag="t1")
                    nc.vector.tensor_tensor(t1[:, :], ghn[:, :],
                                            rz2[:, 0:128], op=ALU.mult)
                    t2 = lp.tile([128, 128], F32, tag="t2")
                    nc.vector.tensor_tensor(t2[:, :], gin[:, :], t1[:, :],
                                            op=ALU.add)
                    n_sb = lp.tile([128, 128], F16, tag="n")
                    nc.scalar.activation(n_sb[:, :], t2[:, :], AF.Tanh)
                    d_sb = lp.tile([128, 128], F32, tag="d")
                    nc.vector.tensor_tensor(d_sb[:, :], h_sb[:, :],
                                            n_sb[:, :], op=ALU.subtract)
                    zd_sb = lp.tile([128, 128], F32, tag="zd")
                    nc.vector.tensor_tensor(zd_sb[:, :], rz2[:, 128:256],
                                            d_sb[:, :], op=ALU.mult)
                    hc_sb = lp.tile([128, 128], F16, tag="hc")
                    nc.vector.tensor_tensor(hc_sb[:, :], n_sb[:, :],
                                            zd_sb[:, :], op=ALU.add)

                    g_ps = lps.tile([128, 128], F32, tag="g")
                    if not no_bmm:
                        for b in range(BC):
                            for it in range(KT):
                                col = it * BC + b
                                for jt in range(KT):
                                    w0 = (b * KT + jt) * H + it * 128
                                    nc.tensor.matmul(
                                        g_ps[:, col:col + 1],
                                        wt_sb[:, w0:w0 + 128],
                                        hc_sb[:, jt * BC + b:jt * BC + b + 1],
                                        start=(jt == 0), stop=(jt == KT - 1))
                    else:
                        nc.tensor.matmul(g_ps[:, :], wt_sb[:, 0:128],
                                         hc_sb[:, :], start=True, stop=True)
                    g2 = lp.tile([128, 128], F32, tag="g2")
                    nc.vector.tensor_tensor(g2[:, :], g_ps[:, :],
                                            bias_sb[:, :], op=ALU.add)
                    nc.scalar.activation(h_sb[:, :], g2[:, :], AF.Tanh)
                    nc.sync.dma_start(out=nh_re_st[bass.ds(i, 1), uu, :],
                                      in_=h_sb[:, :])

        # ---- post pass: decoder, emission, log-likelihood, KL ----
        ROWS = T * BC
        NCH = (ROWS + 509) // 510
        with tc.tile_pool(name="post", bufs=1) as pp, \
             tc.tile_pool(name="postps", bufs=2, space=PSUM) as pps:
            dw_sb = pp.tile([128, KT * H], F16)
            db_sb = pp.tile([128, KT], F32)
            emw_sb = pp.tile([128, KT * OUT], F16)
            evw_sb = pp.tile([128, KT * OUT], F16)
            evb_sb = pp.tile([OUT, 1], F32)
            xt_sb = pp.tile([OUT, SM1 * BC], F32)
            acc_q = pp.tile([OUT, NCH], F32)
            acc_lv = pp.tile([OUT, NCH], F32)
            out_sb = pp.tile([1, 2], F32)
            nc.sync.dma_start(out=dw_sb[:, :], in_=dw_in.ap()[:, :])
            nc.sync.dma_start(out=db_sb[:, :], in_=db_in.ap()[:, :])
            nc.sync.dma_start(out=emw_sb[:, :], in_=emw_in.ap()[:, :])
            nc.sync.dma_start(out=evw_sb[:, :], in_=evw_in.ap()[:, :])
            nc.sync.dma_start(out=evb_sb[:, :], in_=evb_in.ap()[:, :])
            nc.sync.dma_start(out=xt_sb[:, :T * BC], in_=xt_in.ap()[:, :T * BC])

            nh_t = []
            nh_re = nh_dram.ap().rearrange("t (p kt b) -> p kt t b",
                                           p=128, kt=KT, b=BC)
            for kt in range(KT):
                t_ = pp.tile([128, ROWS], F16, tag=f"nh{kt}")
                nc.sync.dma_start(out=t_[:, :],
                                  in_=nh_re[:, kt, :T, :])
                nh_t.append(t_)

            p2_ctx = tc.tile_pool(name="post2", bufs=2)
            p2 = p2_ctx.__enter__()
            for rc in range(NCH):
                r0 = rc * 510
                rn = min(510, ROWS - r0)
                dec_t = []
                for mt in range(KT):
                    ps = pps.tile([128, 510], F32, tag="decp")
                    for kt in range(KT):
                        nc.tensor.matmul(
                            ps[:, :rn],
                            dw_sb[:, kt * H + mt * 128: kt * H + (mt + 1) * 128],
                            nh_t[kt][:, r0:r0 + rn],
                            start=(kt == 0), stop=(kt == KT - 1))
                    dt_ = p2.tile([128, 510], F16, tag=f"dec{mt}")
                    nc.scalar.activation(dt_[:, :rn], ps[:, :rn], AF.Relu,
                                         bias=db_sb[:, mt:mt + 1])
                    dec_t.append(dt_)
                m_ps = pps.tile([OUT, 510], F32, tag="mp")
                v_ps = pps.tile([OUT, 510], F32, tag="vp")
                for kt in range(KT):
                    nc.tensor.matmul(m_ps[:, :rn],
                                     emw_sb[:, kt * OUT:(kt + 1) * OUT],
                                     dec_t[kt][:, :rn],
                                     start=(kt == 0), stop=(kt == KT - 1))
                for kt in range(KT):
                    nc.tensor.matmul(v_ps[:, :rn],
                                     evw_sb[:, kt * OUT:(kt + 1) * OUT],
                                     dec_t[kt][:, :rn],
                                     start=(kt == 0), stop=(kt == KT - 1))
                # e = exp(-(v + evb)) ; d = xt' - m ; q = d*d*e
                e_sb = p2.tile([OUT, 510], F32, tag="e")
                nc.scalar.activation(e_sb[:, :rn], v_ps[:, :rn], AF.Exp,
                                     bias=evb_sb[:, 0:1], scale=-1.0)
                d_sb2 = p2.tile([OUT, 510], F32, tag="dd")
                nc.vector.tensor_tensor(d_sb2[:, :rn], xt_sb[:, r0:r0 + rn],
                                        m_ps[:, :rn], op=ALU.subtract)
                d2_sb = p2.tile([OUT, 510], F32, tag="d2")
                nc.vector.tensor_tensor(d2_sb[:, :rn], d_sb2[:, :rn],
                                        d_sb2[:, :rn], op=ALU.mult)
                q_sb = p2.tile([OUT, 510], F32, tag="q")
                nc.vector.scalar_tensor_tensor(
                    q_sb[:, :rn], d2_sb[:, :rn], 1.0, e_sb[:, :rn],
                    op0=ALU.mult, op1=ALU.mult,
                    accum_out=acc_q[:, rc:rc + 1])
                nc.vector.tensor_reduce(acc_lv[:, rc:rc + 1], v_ps[:, :rn],
                                        axis=mybir.AxisListType.X, op=ALU.add)

            p2_ctx.__exit__(None, None, None)

            # KL on every core (identical) in fp32
            acc_kl = pp.tile([128, KT], F32)
            p2_ctx = tc.tile_pool(name="klp", bufs=1)
            p2 = p2_ctx.__enter__()
            for it in range(KT):
                lv_t = p2.tile([128, H + 1], F32, tag="klv")
                pm_t = p2.tile([128, H + 1], F32, tag="kpm")
                pl_t = p2.tile([128, H + 1], F32, tag="kpl")
                mu_t = p2.tile([128, H + 1], F32, tag="kmu")
                sl = slice(it * 128, (it + 1) * 128)
                nc.sync.dma_start(out=lv_t[:, :], in_=kllv_in.ap()[sl, :])
                nc.sync.dma_start(out=pm_t[:, :], in_=klpm_in.ap()[sl, :])
                nc.sync.dma_start(out=pl_t[:, :], in_=klpl_in.ap()[sl, :])
                nc.sync.dma_start(out=mu_t[:, :], in_=wmu_in.ap()[sl, :])
                eq = p2.tile([128, H + 1], F32, tag="keq")
                nc.scalar.activation(eq[:, :], lv_t[:, :], AF.Exp)
                ep = p2.tile([128, H + 1], F32, tag="kep")
                nc.scalar.activation(ep[:, :], pl_t[:, :], AF.Exp, scale=-1.0)
                dmu = p2.tile([128, H + 1], F32, tag="kdm")
                nc.vector.tensor_tensor(dmu[:, :], mu_t[:, :], pm_t[:, :],
                                        op=ALU.subtract)
                d2m = p2.tile([128, H + 1], F32, tag="kd2")
                nc.vector.tensor_tensor(d2m[:, :], dmu[:, :], dmu[:, :],
                                        op=ALU.mult)
                s_ = p2.tile([128, H + 1], F32, tag="ks")
                nc.vector.tensor_tensor(s_[:, :], eq[:, :], d2m[:, :],
                                        op=ALU.add)
                t1_ = p2.tile([128, H + 1], F32, tag="kt1")
                nc.vector.tensor_tensor(t1_[:, :], s_[:, :], ep[:, :],
                                        op=ALU.mult)
                v1_ = p2.tile([128, H + 1], F32, tag="kv1")
                nc.vector.tensor_tensor(v1_[:, :], pl_t[:, :], lv_t[:, :],
                                        op=ALU.subtract)
                w_ = p2.tile([128, H + 1], F32, tag="kw")
                nc.vector.tensor_tensor(w_[:, :], v1_[:, :], t1_[:, :],
                                        op=ALU.add)
                kls = p2.tile([128, H + 1], F32, tag="kls")
                nc.vector.scalar_tensor_tensor(
                    kls[:, :], w_[:, :], -1.0, w_[:, :],
                    op0=ALU.add, op1=ALU.bypass,
                    accum_out=acc_kl[:, it:it + 1])

            p2_ctx.__exit__(None, None, None)

            # reductions to scalars
            s1a = pp.tile([OUT, 1], F32)
            nc.vector.tensor_reduce(s1a[:, :], acc_q[:, :],
                                    axis=mybir.AxisListType.X, op=ALU.add)
            s2a = pp.tile([OUT, 1], F32)
            nc.vector.tensor_reduce(s2a[:, :], acc_lv[:, :],
                                    axis=mybir.AxisListType.X, op=ALU.add)
            ka = pp.tile([128, 1], F32)
            nc.vector.tensor_reduce(ka[:, :], acc_kl[:, :],
                                    axis=mybir.AxisListType.X, op=ALU.add)
            s1 = pp.tile([1, 1], F32)
            nc.gpsimd.tensor_reduce(s1[:, :], s1a[:, :],
                                    axis=mybir.AxisListType.C, op=ALU.add)
            s2 = pp.tile([1, 1], F32)
            nc.gpsimd.tensor_reduce(s2[:, :], s2a[:, :],
                                    axis=mybir.AxisListType.C, op=ALU.add)
            sb_ = pp.tile([1, 1], F32)
            nc.gpsimd.tensor_reduce(sb_[:, :], evb_sb[:, :],
                                    axis=mybir.AxisListType.C, op=ALU.add)
            k1 = pp.tile([1, 1], F32)
            nc.gpsimd.tensor_reduce(k1[:, :], ka[:, :],
                                    axis=mybir.AxisListType.C, op=ALU.add)
            # LL = -0.5*(S1 + S2 + ROWS*SB + LOG2PI*NE)
            ne = float(ROWS * OUT)
            u1 = pp.tile([1, 1], F32)
            nc.vector.tensor_tensor(u1[:, :], s1[:, :], s2[:, :], op=ALU.add)
            u2 = pp.tile([1, 1], F32)
            nc.vector.scalar_tensor_tensor(u2[:, :], sb_[:, :], float(ROWS),
                                           u1[:, :], op0=ALU.mult, op1=ALU.add)
            u3 = pp.tile([1, 1], F32)
            nc.vector.scalar_tensor_tensor(u3[:, :], u2[:, :],
                                           LOG2PI * ne, u2[:, :],
                                           op0=ALU.add, op1=ALU.bypass)
            nc.vector.tensor_scalar_mul(out_sb[:, 0:1], u3[:, :], -0.5)
            nc.vector.tensor_scalar_mul(out_sb[:, 1:2], k1[:, :], 0.5)
            nc.sync.dma_start(out=out_ext.ap()[:, :], in_=out_sb[:, :])
        if _rep is not None:
            _rep.__exit__(None, None, None)

    return nc


# ---------------------------------------------------------------------------
def prep_inputs(X, A, W_eps, W_mu, W_logvar, prior_W_mu, prior_W_logvar,
                enc_w1, enc_b1, enc_w2, enc_b2,
                gru_w_ih, gru_w_hh, gru_b_ih, gru_b_hh,
                dec_w, dec_b, em_w, em_b, ev_w, ev_b, T=SM1):
    f32 = np.float32
    X = np.asarray(X, f32)
    A = np.asarray(A, f32)
    W_eps = np.asarray(W_eps, f32)
    W_mu = np.asarray(W_mu, f32)
    W_logvar = np.asarray(W_logvar, f32)
    sig = np.exp(0.5 * W_logvar).astype(f32)
    w_hh = np.asarray(gru_w_hh, f32)
    w_ih = np.asarray(gru_w_ih, f32)
    b_ih = np.asarray(gru_b_ih, f32)
    b_hh = np.asarray(gru_b_hh, f32)

    whh8 = np.ascontiguousarray(
        w_hh.T.reshape(KT, 128, MT3, 128).transpose(1, 0, 2, 3)
        .reshape(128, KT * MT3 * 128)).astype(NP_F8)
    wih_aug = np.concatenate([
        w_ih.T,
        np.concatenate([(b_ih + b_hh)[:2 * H], b_ih[2 * H:]])[None, :],
    ], axis=0)
    wih8 = np.ascontiguousarray(wih_aug).astype(NP_F8)
    bhhn8 = np.ascontiguousarray(b_hh[2 * H:][None, :]).astype(NP_F8)
    ones8 = np.ones((1, BC), NP_F8)

    ew1 = np.ascontiguousarray(np.asarray(enc_w1, f32).T).astype(np.float16)
    eb1 = np.ascontiguousarray(np.asarray(enc_b1, f32).reshape(KT, 128).T)
    ew2l = np.ascontiguousarray(
        np.asarray(enc_w2, f32).T.reshape(KT, 128, KT, 128)
        .transpose(1, 0, 2, 3).reshape(128, KT * H)).astype(np.float16)
    eb2 = np.ascontiguousarray(np.asarray(enc_b2, f32).reshape(KT, 128).T)
    dwl = np.ascontiguousarray(
        np.asarray(dec_w, f32).T.reshape(KT, 128, KT, 128)
        .transpose(1, 0, 2, 3).reshape(128, KT * H)).astype(np.float16)
    db = np.ascontiguousarray(np.asarray(dec_b, f32).reshape(KT, 128).T)
    emwl = np.ascontiguousarray(
        np.asarray(em_w, f32).T.reshape(KT, 128, OUT)
        .transpose(1, 0, 2).reshape(128, KT * OUT)).astype(np.float16)
    evwl = np.ascontiguousarray(
        np.asarray(ev_w, f32).T.reshape(KT, 128, OUT)
        .transpose(1, 0, 2).reshape(128, KT * OUT)).astype(np.float16)
    evb = np.asarray(ev_b, f32).reshape(OUT, 1)
    em_b = np.asarray(em_b, f32)
    ident = np.eye(128, dtype=f32)

    shared = dict(
        wmu32=np.ascontiguousarray(W_mu),
        sig32=np.ascontiguousarray(sig),
        whh8=whh8, wih8=wih8, bhhn8=bhhn8, ones8=ones8,
        ew1=ew1, eb1=eb1, ew2l=ew2l, eb2=eb2,
        dwl=dwl, db=db, emwl=emwl, evwl=evwl, evb=evb,
        ident=ident,
        kl_lv=np.ascontiguousarray(W_logvar),
        kl_pm=np.ascontiguousarray(np.asarray(prior_W_mu, f32)),
        kl_pl=np.ascontiguousarray(np.asarray(prior_W_logvar, f32)),
    )

    in_maps = []
    for c in range(NC):
        bs = c * BC
        m = dict(shared)
        m["eps"] = np.ascontiguousarray(W_eps[bs:bs + BC])
        a_tr = A[:, bs:bs + BC, :].transpose(2, 0, 1)  # [8, T, BC]
        a_aug = np.concatenate(
            [a_tr, np.ones((1, SM1, BC), f32)], axis=0)
        m["a_t"] = np.ascontiguousarray(a_aug).astype(NP_F8)
        m["x0"] = np.ascontiguousarray(X[0, bs:bs + BC, :].T).astype(np.float16)
        m["xt"] = np.ascontiguousarray(
            (X[1:, bs:bs + BC, :] - em_b[None, None, :])
            .transpose(2, 0, 1).reshape(OUT, SM1 * BC))
        in_maps.append(m)
    return in_maps


_NC_CACHE = {}


def kernel(X, A, W_eps, W_mu, W_logvar, prior_W_mu, prior_W_logvar,
           enc_w1, enc_b1, enc_w2, enc_b2,
           gru_w_ih, gru_w_hh, gru_b_ih, gru_b_hh,
           dec_w, dec_b, em_w, em_b, ev_w, ev_b, N):
    in_maps = prep_inputs(X, A, W_eps, W_mu, W_logvar, prior_W_mu,
                          prior_W_logvar, enc_w1, enc_b1, enc_w2, enc_b2,
                          gru_w_ih, gru_w_hh, gru_b_ih, gru_b_hh,
                          dec_w, dec_b, em_w, em_b, ev_w, ev_b)
    if SM1 not in _NC_CACHE:
        _NC_CACHE[SM1] = build_nc(SM1)
    nc = _NC_CACHE[SM1]
    res = run_bass_kernel_spmd(nc, in_maps, core_ids=list(range(NC)))
    outs = [r["out"] for r in res.results]
    LL = float(sum(o[0, 0] for o in outs))
    KL = float(outs[0][0, 1])
    LL_n = LL / B
    KL_n = KL / (SM1 * B)
    return np.array([LL_n - KL_n, LL_n, KL_n], dtype=np.float32)
